# revision 25
# baseline (speedup 1.0000x reference)
"""Trainium2 Bass kernel for the Equiformer-style GNN regressor.

Strategy (8 NeuronCores, SPMD), v2 — bf16 rewrite of the fp32 baseline:
  - Nodes are permuted on host (LPT on in-degree) so each of the 80
    node-tiles (8 cores x 10 tiles of 128) owns 125 real nodes and at most
    4096 incoming edges -> cpt=8 chunks of 512 edge slots per tile.
  - Per layer, a DRAM node table ([10240, 640] bf16, 4 head-blocks of 144:
    [s 32 | v 48 | t 40 | sv 16 | st 8] + pad) holds per-node projected
    quantities. Edge phase gathers rows edge-major, computes radial
    projections DIRECTLY edge-major on PE (lhsT = radial-MLP weights chunk,
    rhs = packed per-layer wwall) into PSUM -- no transposes -- and forms
    messages with DVE ops reading PSUM. Aggregation per dst node-tile is a
    single pair of one-hot bf16 matmuls per 128-edge sub (den + 480-wide
    strided message rhs) accumulating into one PSUM bank.
  - RBF basis (layer-independent) is computed once in fp32 (numerically
    sensitive quadratic form) and kept SBUF-resident in bf16.
  - Update phase (fp32): out-projections + residual + equivariant norms +
    next-layer projections; new bf16 table rows AllGathered across cores.
  - Final readout (per-node energies) is DMA'd out; per-graph segment-sum +
    Linear(1,1) runs on host.
"""
import math
import sys
import types
from contextlib import ExitStack
from dataclasses import dataclass

import numpy as np
import ml_dtypes

import concourse.bacc as bacc
import concourse.bass as bass
import concourse.tile as tile
from concourse import mybir
from concourse.bass_utils import run_bass_kernel_spmd

F32 = mybir.dt.float32
BF16 = mybir.dt.bfloat16
AF = mybir.ActivationFunctionType
OP = mybir.AluOpType
BF = ml_dtypes.bfloat16

# ---------------- problem constants (hardcoded per spec) ----------------
N, E, G, L = 10000, 320000, 32, 6
C0, C1, C2, H, NB, RAD, FD, T = 128, 64, 32, 4, 128, 64, 512, 1
MAXR = 5.0
EPS = 1e-6
NCORE = 8
P = 128

BLK = 144         # per-head block width in table rows and rp banks
GW = 4 * BLK      # used table row width (576)
GWP = 640         # padded table row width
AW = 128 + 2 * BLK   # rp bank A width: R_sa + blocks 0,1 (416)
BW = 2 * BLK         # rp bank B width: blocks 2,3 (288)
# offsets inside a head block
O_S, O_V, O_T, O_SV, O_ST = 0, 32, 80, 120, 136
HB = 148          # hh per-head block: [ex 1 | 144 msg block | pad 3]
HHW = 4 * HB      # 592

EM_DST, EM_MASK, EM_SH1, EM_SH2, EMW = 0, 1, 2, 5, 12


@dataclass
class Cfg:
    ncore: int = NCORE
    npc: int = 1280          # padded nodes per core (multiple of 128)
    cpt: int = 8             # chunks (512 slots) per node-tile
    layers: int = L
    nn: int = N              # real node count

    @property
    def ntile(self):
        return self.npc // P

    @property
    def np_total(self):
        return self.npc * self.ncore

    @property
    def slots(self):
        return self.ntile * self.cpt * 512


# ---------------- host-side packing helpers ----------------

def g_col_maps():
    """Column index maps for the 640-wide node-table row layout."""
    ps = np.zeros(C0, np.int64)
    psv = np.zeros(C1, np.int64)
    pst = np.zeros(C2, np.int64)
    pv = np.zeros((C1, 3), np.int64)
    pt = np.zeros((C2, 5), np.int64)
    for c in range(C0):
        h, j = divmod(c, 32)
        ps[c] = BLK * h + O_S + j
    for c in range(C1):
        h, j = divmod(c, 16)
        psv[c] = BLK * h + O_SV + j
        for i in range(3):
            pv[c, i] = BLK * h + O_V + 16 * i + j
    for c in range(C2):
        h, j = divmod(c, 8)
        pst[c] = BLK * h + O_ST + j
        for m in range(5):
            pt[c, m] = BLK * h + O_T + 8 * m + j
    return ps, psv, pst, pv, pt


G_PS, G_PSV, G_PST, G_PV, G_PT = g_col_maps()


def pack_node_table(P_s, P_sv, P_st, Pv, Pt):
    """[n,C0],[n,C1],[n,C2],[n,C1,3],[n,C2,5] -> [n, 640] bf16."""
    n = P_s.shape[0]
    out = np.zeros((n, GWP), np.float32)
    out[:, G_PS] = P_s
    out[:, G_PSV] = P_sv
    out[:, G_PST] = P_st
    out[:, G_PV.reshape(-1)] = Pv.reshape(n, -1)
    out[:, G_PT.reshape(-1)] = Pt.reshape(n, -1)
    return out.astype(BF)


def pack_wwall(Ww_s, Ww_v, Ww_t, Ww_vv, Ww_tt, attn_a):
    """Per-layer radial-projection weights -> bankA [RAD,416], bankB [RAD,288]."""
    full = np.zeros((RAD, 128 + 4 * BLK), np.float32)
    for c in range(C0):
        h, j = divmod(c, 32)
        full[:, c] = Ww_s[:, c] * attn_a[h, j]
    for h in range(H):
        base = 128 + BLK * h
        full[:, base + O_S:base + O_S + 32] = Ww_s[:, 32 * h:32 * h + 32]
        for c in range(16):
            for i in range(3):
                full[:, base + O_V + 16 * i + c] = Ww_vv[:, 16 * h + c]
        for c in range(8):
            for m in range(5):
                full[:, base + O_T + 8 * m + c] = Ww_tt[:, 8 * h + c]
        full[:, base + O_SV:base + O_SV + 16] = Ww_v[:, 16 * h:16 * h + 16]
        full[:, base + O_ST:base + O_ST + 8] = Ww_t[:, 8 * h:8 * h + 8]
    return full[:, :AW].astype(BF), full[:, AW:].astype(BF)


def balance_nodes(edst, cfg: Cfg):
    """LPT-assign nodes to the 80 (core,tile) bins, 125 real nodes each.
    Returns pid_of[node] (padded id) and the inverse slot list."""
    nbin = cfg.ncore * cfg.ntile
    per_bin = cfg.nn // nbin
    assert per_bin * nbin == cfg.nn
    deg = np.bincount(edst, minlength=cfg.nn)
    order = np.argsort(-deg, kind="stable")
    bin_edges = np.zeros(nbin, np.int64)
    bin_count = np.zeros(nbin, np.int64)
    pid_of = np.zeros(cfg.nn, np.int64)
    import heapq
    heap = [(0, b) for b in range(nbin)]
    heapq.heapify(heap)
    for nd in order:
        while True:
            e, b = heapq.heappop(heap)
            if bin_count[b] < per_bin:
                break
        c, t = divmod(b, cfg.ntile)
        pid_of[nd] = c * cfg.npc + t * P + bin_count[b]
        bin_count[b] += 1
        bin_edges[b] += deg[nd]
        if bin_count[b] < per_bin:
            heapq.heappush(heap, (bin_edges[b], b))
    assert bin_edges.max() <= cfg.cpt * 512, f"tile overflow {bin_edges.max()}"
    return pid_of


def host_preprocess(inp, cfg: Cfg):
    """Build all per-core device input arrays + schedule."""
    npc, ncore, ntile = cfg.npc, cfg.ncore, cfg.ntile

    pos = np.asarray(inp["pos"], np.float32)
    node_atom = np.asarray(inp["node_atom"]).astype(np.int64)
    esrc = np.asarray(inp["edge_src"]).astype(np.int64)
    edst = np.asarray(inp["edge_dst"]).astype(np.int64)

    pid_of = balance_nodes(edst, cfg)
    src_p = pid_of[esrc]
    dst_p = pid_of[edst]

    # geometry (match reference formulas, f32)
    rel = pos[edst] - pos[esrc]
    d2 = (rel * rel).sum(-1) + np.float32(EPS)
    d = np.sqrt(d2)
    u = rel / d[:, None]
    s3, s5, s15 = [np.float32(np.sqrt(x)) for x in (3.0, 5.0, 15.0)]
    sh1 = s3 * u
    x_, y_, z_ = u[:, 0], u[:, 1], u[:, 2]
    sh2 = np.stack(
        [s15 * x_ * y_, s15 * y_ * z_, np.float32(0.5) * s5 * (3 * z_ * z_ - 1.0),
         s15 * x_ * z_, np.float32(0.5) * s15 * (x_ * x_ - y_ * y_)], -1)

    # per-(core,tile) edge lists
    tile_of_edge = (dst_p // npc) * ntile + (dst_p % npc) // P
    order = np.argsort(tile_of_edge, kind="stable")
    counts = np.bincount(tile_of_edge, minlength=ncore * ntile)
    assert counts.max() <= cfg.cpt * 512
    starts = np.zeros(ncore * ntile + 1, np.int64)
    np.cumsum(counts, out=starts[1:])

    S = cfg.slots
    per_core = []
    for c in range(ncore):
        slot_src = np.zeros(S, np.int64)
        slot_dstrel = np.zeros(S, np.float32)
        slot_mask = np.zeros(S, np.float32)
        slot_d2 = np.ones(S, np.float32)
        slot_d = np.ones(S, np.float32)
        slot_sh1 = np.zeros((S, 3), np.float32)
        slot_sh2 = np.zeros((S, 5), np.float32)
        for t in range(ntile):
            gt = c * ntile + t
            eids = order[starts[gt]:starts[gt + 1]]
            base = t * cfg.cpt * 512
            k = len(eids)
            sl = slice(base, base + k)
            slot_src[sl] = src_p[eids]
            slot_dstrel[sl] = (dst_p[eids] - (c * npc + t * P)).astype(np.float32)
            slot_mask[sl] = 1.0
            slot_d2[sl] = d2[eids]
            slot_d[sl] = d[eids]
            slot_sh1[sl] = sh1[eids]
            slot_sh2[sl] = sh2[eids]

        nsub = S // P
        em = np.zeros((P, nsub, EMW), np.float32)
        j = np.arange(S)
        em[j % P, j // P, EM_DST] = slot_dstrel
        em[j % P, j // P, EM_MASK] = slot_mask
        em[j % P, j // P, EM_SH1:EM_SH1 + 3] = slot_sh1
        em[j % P, j // P, EM_SH2:EM_SH2 + 5] = slot_sh2
        geom = np.ones((3, S), np.float32)
        geom[0] = slot_d2
        geom[1] = slot_d
        idx_em = np.zeros((P, nsub), np.int32)
        idx_em[j % P, j // P] = slot_src.astype(np.int32)
        per_core.append(dict(em=em, geom=geom, idxw=idx_em))

    # ---- weights ----
    wd = {}
    centers = np.linspace(0.0, MAXR, NB).astype(np.float32)
    width = np.float32(MAXR / NB)
    wq = np.zeros((3, NB), np.float32)
    wq[0] = -0.5 / width**2
    wq[1] = centers / width**2
    wq[2] = -0.5 * centers**2 / width**2
    wd["wq"] = wq
    wd["wrad1"] = np.asarray(inp["Wrad1"], np.float32).astype(BF)
    wd["wrad2"] = np.asarray(inp["Wrad2"], np.float32).astype(BF)
    wd["brad1"] = np.asarray(inp["brad1"], np.float32).reshape(cfg.layers, RAD, 1)
    wd["brad2"] = np.asarray(inp["brad2"], np.float32).reshape(cfg.layers, RAD, 1)
    wwA, wwB = [], []
    for l in range(cfg.layers):
        a, b = pack_wwall(inp["Ww_s"][l], inp["Ww_v"][l], inp["Ww_t"][l],
                          inp["Ww_vv"][l], inp["Ww_tt"][l], inp["attn_a"][l])
        wwA.append(a)
        wwB.append(b)
    wd["wwa"] = np.stack(wwA)
    wd["wwb"] = np.stack(wwB)
    wd["wo_s"] = np.asarray(inp["Wo_s"], np.float32).astype(BF)
    wd["wo_v"] = np.asarray(inp["Wo_v"], np.float32).astype(BF)
    wd["wo_t"] = np.asarray(inp["Wo_t"], np.float32).astype(BF)
    wd["ws_src"] = np.asarray(inp["Ws_src"], np.float32).astype(BF)
    wd["ws_v"] = np.asarray(inp["Ws_v"], np.float32).astype(BF)
    wd["ws_t"] = np.asarray(inp["Ws_t"], np.float32).astype(BF)
    wd["wv_v"] = np.asarray(inp["Wv_v"], np.float32).astype(BF)
    wd["wt_t"] = np.asarray(inp["Wt_t"], np.float32).astype(BF)
    rep = lambda a: np.broadcast_to(a[:, None, :], (a.shape[0], P, a.shape[1])).copy()
    wd["lngs"] = rep(np.asarray(inp["g_s"], np.float32)).astype(BF)
    wd["lnbs"] = rep(np.asarray(inp["b_s"], np.float32)).astype(BF)
    wd["lngv"] = rep(np.asarray(inp["g_v"], np.float32)).astype(BF)
    wd["lngt"] = rep(np.asarray(inp["g_t"], np.float32)).astype(BF)
    wd["wfeat"] = np.asarray(inp["W_feat"], np.float32).astype(BF)
    wd["bfeatp"] = np.asarray(inp["b_feat"], np.float32).reshape(4, 128).T.copy()
    wd["wout1p"] = np.asarray(inp["W_out1"], np.float32).reshape(4, 128).T.copy().astype(BF)
    wd["nidx"] = np.tile(np.arange(P, dtype=np.float32), (P, 1)).astype(BF)
    wd["identb"] = np.eye(P, dtype=np.float32).astype(BF)

    # ---- initial node table (layer 0 projections) + s0 feature-major ----
    s0 = np.asarray(inp["atom_emb"], np.float32)[node_atom]     # [N, C0]
    s0p = np.zeros((cfg.np_total, C0), np.float32)
    s0p[pid_of] = s0
    ntab0 = pack_node_table(
        s0p @ inp["Ws_src"][0], s0p @ inp["Ws_v"][0], s0p @ inp["Ws_t"][0],
        np.zeros((cfg.np_total, C1, 3), np.float32),
        np.zeros((cfg.np_total, C2, 5), np.float32))

    in_maps = []
    for c in range(ncore):
        m = dict(per_core[c])
        m["ntab0"] = ntab0
        m["s0fm"] = s0p[c * npc:(c + 1) * npc].T.copy().astype(BF)   # [C0, npc]
        for k, v in wd.items():
            m[k] = v
        in_maps.append(m)
    return in_maps, pid_of


# ---------------- device program ----------------

def reap(sliced: bass.AP, dims) -> bass.AP:
    """Rebuild the free-dims of a (narrow) sliced AP with explicit
    [step, count] pairs, keeping its partition dim and offset."""
    return bass.AP(sliced.tensor, sliced.offset,
                   [list(sliced.ap[0])] + [[int(s), int(c)] for s, c in dims])


def build_program(cfg: Cfg):
    nc = bacc.Bacc("TRN2", target_bir_lowering=False, debug=False,
                   enable_asserts=True, num_devices=cfg.ncore)
    npc, ntile, cpt = cfg.npc, cfg.ntile, cfg.cpt
    S = cfg.slots
    nsub = S // P
    NPT = cfg.np_total
    LYR = cfg.layers

    dp = nc.declare_dram_parameter
    t_ntab0 = dp("ntab0", [NPT, GWP], BF16, isOutput=False)
    t_s0fm = dp("s0fm", [C0, npc], BF16, isOutput=False)
    t_em = dp("em", [P, nsub, EMW], F32, isOutput=False)
    t_geom = dp("geom", [3, S], F32, isOutput=False)
    t_idxw = dp("idxw", [P, nsub], mybir.dt.int32, isOutput=False)
    t_wq = dp("wq", [3, NB], F32, isOutput=False)
    t_wrad1 = dp("wrad1", [LYR, NB, RAD], BF16, isOutput=False)
    t_wrad2 = dp("wrad2", [LYR, RAD, RAD], BF16, isOutput=False)
    t_brad1 = dp("brad1", [LYR, RAD, 1], F32, isOutput=False)
    t_brad2 = dp("brad2", [LYR, RAD, 1], F32, isOutput=False)
    t_wwa = dp("wwa", [LYR, RAD, AW], BF16, isOutput=False)
    t_wwb = dp("wwb", [LYR, RAD, BW], BF16, isOutput=False)
    t_wo_s = dp("wo_s", [LYR, C0, C0], BF16, isOutput=False)
    t_wo_v = dp("wo_v", [LYR, C1, C1], BF16, isOutput=False)
    t_wo_t = dp("wo_t", [LYR, C2, C2], BF16, isOutput=False)
    t_ws_src = dp("ws_src", [LYR, C0, C0], BF16, isOutput=False)
    t_ws_v = dp("ws_v", [LYR, C0, C1], BF16, isOutput=False)
    t_ws_t = dp("ws_t", [LYR, C0, C2], BF16, isOutput=False)
    t_wv_v = dp("wv_v", [LYR, C1, C1], BF16, isOutput=False)
    t_wt_t = dp("wt_t", [LYR, C2, C2], BF16, isOutput=False)
    t_lngs = dp("lngs", [LYR, P, C0], BF16, isOutput=False)
    t_lnbs = dp("lnbs", [LYR, P, C0], BF16, isOutput=False)
    t_lngv = dp("lngv", [LYR, P, C1], BF16, isOutput=False)
    t_lngt = dp("lngt", [LYR, P, C2], BF16, isOutput=False)
    t_wfeat = dp("wfeat", [C0, FD], BF16, isOutput=False)
    t_bfeatp = dp("bfeatp", [P, 4], F32, isOutput=False)
    t_wout1p = dp("wout1p", [P, 4], BF16, isOutput=False)
    t_nidx = dp("nidx", [P, P], BF16, isOutput=False)
    t_identb = dp("identb", [P, P], BF16, isOutput=False)
    t_nodee = dp("node_e", [npc], F32, isOutput=True)

    own = [nc.dram_tensor(f"own{l}", [npc, GWP], BF16) for l in range(LYR - 1)]
    ntab = [nc.dram_tensor(f"ntab{l + 1}", [NPT, GWP], BF16, addr_space="Shared")
            for l in range(LYR - 1)]

    with tile.TileContext(nc) as tc, ExitStack() as ctx:
        pool1 = ctx.enter_context(tc.tile_pool(name="const", bufs=1))
        poolL = ctx.enter_context(tc.tile_pool(name="layerw", bufs=1))
        poolT = ctx.enter_context(tc.tile_pool(name="tilec", bufs=2))
        poolg = ctx.enter_context(tc.tile_pool(name="gath", bufs=2))
        poole = ctx.enter_context(tc.tile_pool(name="edge", bufs=2))
        poolx = ctx.enter_context(tc.tile_pool(name="edge1", bufs=2))
        poolu = ctx.enter_context(tc.tile_pool(name="upd", bufs=1))
        psA = ctx.enter_context(tc.tile_pool(name="psA", bufs=2, space="PSUM"))
        psRP = ctx.enter_context(tc.tile_pool(name="psRP", bufs=2, space="PSUM"))
        psAgg = ctx.enter_context(tc.tile_pool(name="psAgg", bufs=1, space="PSUM"))

        def load1(dram, shape, dtype=F32):
            t = pool1.tile(shape, dtype, tag=dram.name)
            nc.sync.dma_start(out=t[:], in_=dram[:])
            return t

        # resident constants
        wq_t = load1(t_wq, [3, NB])
        nidx_t = load1(t_nidx, [P, P], BF16)
        ident_t = load1(t_identb, [P, P], BF16)
        wfeat_t = load1(t_wfeat, [C0, FD], BF16)
        bfeatp_t = load1(t_bfeatp, [P, 4])
        wout1p_t = load1(t_wout1p, [P, 4], BF16)

        eps_t = pool1.tile([P, 1], F32, tag="epsT")
        nc.vector.memset(eps_t[:], EPS)

        # feature-major stores for own nodes (bf16)
        sfm = pool1.tile([C0, npc], BF16, tag="sfm")
        nc.sync.dma_start(out=sfm[:], in_=t_s0fm[:])
        vfm_t = pool1.tile([C1, 3, npc], BF16, tag="vfm")
        nc.vector.memset(vfm_t[:], 0.0)
        tfm_t = pool1.tile([C2, 5, npc], BF16, tag="tfm")
        nc.vector.memset(tfm_t[:], 0.0)

        def vfm(i):
            return vfm_t[:, i, :]

        def tfm(m):
            return tfm_t[:, m, :]

        # ---- RBF pre-pass: layer-independent, fp32 quadratic form ----
        rbf_all = pool1.tile([NB, S], BF16, tag="rbf_all")
        for k in range(S // 512):
            gsl = poolT.tile([3, 512], F32, tag="geom_c")
            nc.sync.dma_start(out=gsl[:], in_=t_geom[:, k * 512:(k + 1) * 512])
            ps = psA.tile([NB, 512], F32, tag="mmA", space="PSUM")
            nc.tensor.matmul(ps[:], wq_t[:], gsl[:], start=True, stop=True)
            nc.scalar.activation(out=rbf_all[:, k * 512:(k + 1) * 512],
                                 in_=ps[:], func=AF.Exp)

        def loadL(dram, l, p, f, tag, dtype=F32):
            t = poolL.tile([p, f], dtype, tag=tag)
            nc.sync.dma_start(out=t[:], in_=dram[l])
            return t

        def edge_tile(l, t, gsrc, lw, tct):
            """Edge phase for node-tile t of layer l. Returns agg psum tile.

            agg layout: 4 head blocks of AGB: [den 1 | s 32 (| v 48 | t 40)].
            """
            em_s, idx_s, tbase = tct
            last_v = l < LYR - 1
            wlen = BLK if last_v else 32     # msg cols per head block
            agb = 120 if last_v else 32      # aggregated msg cols per block
            agg = psAgg.tile([P, 484], F32, tag="agg", space="PSUM")
            # phase 1: radial MLP for all chunks (batched silus, one table load)
            w_b = poolx.tile([RAD, cpt * 512], BF16, tag="w_b")
            for k in range(cpt):
                sbase = (tbase + k) * 512
                ps1 = psA.tile([RAD, 512], F32, tag="mmA", space="PSUM")
                nc.tensor.matmul(ps1[:], lw["wrad1"][:],
                                 rbf_all[:, sbase:sbase + 512], start=True, stop=True)
                h1 = poolx.tile([RAD, 512], BF16, tag="h1")
                nc.scalar.activation(out=h1[:], in_=ps1[:], func=AF.Silu,
                                     bias=lw["brad1"][:])
                ps2 = psA.tile([RAD, 512], F32, tag="mmA", space="PSUM")
                nc.tensor.matmul(ps2[:], lw["wrad2"][:], h1[:], start=True, stop=True)
                nc.scalar.activation(out=w_b[:, k * 512:(k + 1) * 512], in_=ps2[:],
                                     func=AF.Silu, bias=lw["brad2"][:])
            # phase 2: gather + messages + aggregation
            for k in range(cpt):
                gt = poolg.tile([P, 4, GWP], BF16, tag="gt")
                for sub4 in range(4):
                    nc.gpsimd.indirect_dma_start(
                        out=gt[:, sub4, :], out_offset=None, in_=gsrc[:, :],
                        in_offset=bass.IndirectOffsetOnAxis(
                            ap=idx_s[:, k * 4 + sub4:k * 4 + sub4 + 1], axis=0))
                logit = poole.tile([P, 16], F32, tag="logit")
                ex = poole.tile([P, 16], BF16, tag="ex")
                hh = poole.tile([P, 4, GW], BF16, tag="hh")
                scr = poole.tile([P, 128], F32, tag="scr")
                t2v = poolx.tile([P, 192], BF16, tag="t2v")
                t2t = poolx.tile([P, 160], BF16, tag="t2t")
                for sub in range(4):
                    cs = k * 4 + sub
                    wsl = w_b[:, (k * 4 + sub) * 128:(k * 4 + sub + 1) * 128]
                    # radial projections, direct edge-major: [128e, AW/BW]
                    rpA = psRP.tile([P, AW], F32, tag="rpA", space="PSUM")
                    nc.tensor.matmul(rpA[:], wsl, lw["wwa"][:], start=True, stop=True)
                    rpB = psRP.tile([P, BW], F32, tag="rpB", space="PSUM")
                    if last_v:
                        nc.tensor.matmul(rpB[:], wsl, lw["wwb"][:],
                                         start=True, stop=True)
                    else:
                        # last layer: only R_s of head blocks 2,3 (packed 0:64)
                        nc.tensor.matmul(rpB[:, 0:2 * wlen], wsl,
                                         reap(lw["wwb"][:, 0:1], [(BLK, 2), (1, wlen)]),
                                         start=True, stop=True)
                    # logits: (G_s * R_sa) summed over the 32 head channels
                    nc.vector.tensor_tensor(
                        out=scr[:],
                        in0=reap(gt[:, sub:sub + 1, 0:1], [(BLK, 4), (1, 32)]),
                        in1=rpA[:, 0:128],
                        op=OP.mult)
                    nc.vector.tensor_reduce(
                        out=logit[:, sub * 4:sub * 4 + 4],
                        in_=scr[:].rearrange("p (h c) -> p h c", h=H),
                        axis=mybir.AxisListType.X, op=OP.add)
                    msk = em_s[:, cs, EM_MASK:EM_MASK + 1]
                    exs = slice(sub * 4, sub * 4 + 4)
                    # mask logits so pad-slot exp can't overflow (ex_pad = 1,
                    # killed later by the mask folded into the one-hot)
                    nc.vector.tensor_scalar(out=logit[:, exs], in0=logit[:, exs],
                                            scalar1=msk, scalar2=None, op0=OP.mult)
                    nc.scalar.activation(out=ex[:, exs], in_=logit[:, exs],
                                         func=AF.Exp)
                    # hh msg cols = G * rp   (blocks 0,1 from rpA; 2,3 from rpB)
                    nc.vector.tensor_tensor(
                        out=reap(hh[:, sub:sub + 1, 0:1], [(BLK, 2), (1, wlen)]),
                        in0=reap(gt[:, sub:sub + 1, 0:1], [(BLK, 2), (1, wlen)]),
                        in1=reap(rpA[:, 128:129], [(BLK, 2), (1, wlen)]),
                        op=OP.mult)
                    bstep = BLK if last_v else wlen
                    nc.vector.tensor_tensor(
                        out=reap(hh[:, sub:sub + 1, 2 * BLK:2 * BLK + 1],
                                 [(BLK, 2), (1, wlen)]),
                        in0=reap(gt[:, sub:sub + 1, 2 * BLK:2 * BLK + 1],
                                 [(BLK, 2), (1, wlen)]),
                        in1=reap(rpB[:, 0:1], [(bstep, 2), (1, wlen)]),
                        op=OP.mult)
                    # hh msg *= alpha (broadcast per head block)
                    nc.vector.tensor_tensor(
                        out=reap(hh[:, sub:sub + 1, 0:1], [(BLK, 4), (1, wlen)]),
                        in0=reap(hh[:, sub:sub + 1, 0:1], [(BLK, 4), (1, wlen)]),
                        in1=reap(ex[:, sub * 4:sub * 4 + 1], [(1, 4), (0, wlen)]),
                        op=OP.mult)
                    if last_v:
                        sh1a = em_s[:, cs:cs + 1, EM_SH1:EM_SH1 + 1]
                        sh2a = em_s[:, cs:cs + 1, EM_SH2:EM_SH2 + 1]
                        # T2v = (alpha*A_v) outer sh1 : iter (h, i, c')
                        nc.vector.tensor_tensor(
                            out=t2v[:],
                            in0=reap(hh[:, sub:sub + 1, O_SV:O_SV + 1],
                                     [(BLK, 4), (0, 3), (1, 16)]),
                            in1=reap(sh1a, [(0, 4), (1, 3), (0, 16)]),
                            op=OP.mult)
                        nc.vector.tensor_tensor(
                            out=t2t[:],
                            in0=reap(hh[:, sub:sub + 1, O_ST:O_ST + 1],
                                     [(BLK, 4), (0, 5), (1, 8)]),
                            in1=reap(sh2a, [(0, 4), (1, 5), (0, 8)]),
                            op=OP.mult)
                        # msg_v = T2v + (alpha*Pv*R_vv);  msg_t similarly
                        nc.vector.tensor_tensor(
                            out=reap(hh[:, sub:sub + 1, O_V:O_V + 1],
                                     [(BLK, 4), (16, 3), (1, 16)]),
                            in0=t2v[:],
                            in1=reap(hh[:, sub:sub + 1, O_V:O_V + 1],
                                     [(BLK, 4), (16, 3), (1, 16)]),
                            op=OP.add)
                        nc.vector.tensor_tensor(
                            out=reap(hh[:, sub:sub + 1, O_T:O_T + 1],
                                     [(BLK, 4), (8, 5), (1, 8)]),
                            in0=t2t[:],
                            in1=reap(hh[:, sub:sub + 1, O_T:O_T + 1],
                                     [(BLK, 4), (8, 5), (1, 8)]),
                            op=OP.add)
                    # one-hot with the pad mask folded in, then ONE agg matmul
                    first = (k == 0 and sub == 0)
                    last = (k == cpt - 1 and sub == 3)
                    oh = poole.tile([P, P], BF16, tag="oh")
                    nc.vector.tensor_scalar(out=oh[:], in0=nidx_t[:],
                                            scalar1=em_s[:, cs, EM_DST:EM_DST + 1],
                                            scalar2=msk, op0=OP.is_equal, op1=OP.mult)
                    nc.tensor.matmul(agg[:, 480:484], oh[:], ex[:, exs],
                                     start=first, stop=last, skip_group_check=True)
                    nc.tensor.matmul(
                        agg[:, 0:4 * agb], oh[:],
                        reap(hh[:, sub:sub + 1, 0:1], [(BLK, 4), (1, agb)]),
                        start=False, stop=last, skip_group_check=True)
            return agg

        def transpose_to(src_ap, kparts, ffree):
            """transpose src [kparts, ffree] sbuf bf16 -> psum bf16 [ffree, kparts]"""
            ps = psA.tile([P, P], BF16, tag="mmA", space="PSUM")
            nc.tensor.transpose(ps[:ffree, :kparts], src_ap,
                                ident_t[:kparts, :kparts])
            return ps

        def update_tile(l, t, agg, lw):
            tsl = slice(t * P, (t + 1) * P)
            last_v = l < LYR - 1
            agb = 120 if last_v else 32
            rden = poolu.tile([P, H], F32, tag="rden")
            nc.vector.tensor_scalar(out=rden[:], in0=agg[:, 480:484],
                                    scalar1=1e-9, scalar2=None, op0=OP.add)
            nc.vector.reciprocal(out=rden[:], in_=rden[:])
            # component-major aggnm (bf16): s 0:128 (32h+j), v 128+64i+16h+j,
            # t 320+32m+8h+j -- so the transposes below read contiguous cols
            aggnm = poolu.tile([P, 480], BF16, tag="aggnm")
            for h in range(H):
                nc.vector.tensor_scalar(
                    out=aggnm[:, 32 * h:32 * h + 32],
                    in0=agg[:, agb * h:agb * h + 32],
                    scalar1=rden[:, h:h + 1], scalar2=None, op0=OP.mult)
                if last_v:
                    nc.vector.tensor_scalar(
                        out=reap(aggnm[:, 128 + 16 * h:128 + 16 * h + 1],
                                 [(64, 3), (1, 16)]),
                        in0=reap(agg[:, agb * h + O_V:agb * h + O_V + 1],
                                 [(16, 3), (1, 16)]),
                        scalar1=rden[:, h:h + 1], scalar2=None, op0=OP.mult)
                    nc.vector.tensor_scalar(
                        out=reap(aggnm[:, 320 + 8 * h:320 + 8 * h + 1],
                                 [(32, 5), (1, 8)]),
                        in0=reap(agg[:, agb * h + O_T:agb * h + O_T + 1],
                                 [(8, 5), (1, 8)]),
                        scalar1=rden[:, h:h + 1], scalar2=None, op0=OP.mult)

            # transpose agg to feature-major + out-projections + residual
            psS = transpose_to(aggnm[:, 0:128], P, P)
            afm_s = poolu.tile([P, P], BF16, tag="afm_s")
            nc.scalar.copy(out=afm_s[:], in_=psS[:, :P])
            pso = psA.tile([P, P], F32, tag="mmA", space="PSUM")
            nc.tensor.matmul(pso[:], lw["wo_s"][:], afm_s[:], start=True, stop=True)
            upd_s = poolu.tile([P, P], BF16, tag="upd_s")
            nc.vector.tensor_tensor(out=upd_s[:], in0=sfm[:, tsl], in1=pso[:], op=OP.add)

            upd_v = poolu.tile([C1, 3, P], BF16, tag="upd_v")
            upd_t = poolu.tile([C2, 5, P], BF16, tag="upd_t")
            if last_v:
                for i in range(3):
                    psV = transpose_to(aggnm[:, 128 + 64 * i:128 + 64 * i + 64],
                                       P, C1)
                    afm = poolu.tile([C1, P], BF16, tag="afm_v")
                    nc.scalar.copy(out=afm[:], in_=psV[:C1, :P])
                    psv2 = psA.tile([C1, P], F32, tag="mmA", space="PSUM")
                    nc.tensor.matmul(psv2[:], lw["wo_v"][:], afm[:], start=True, stop=True)
                    nc.vector.tensor_tensor(out=upd_v[:, i, :], in0=vfm(i)[:, tsl],
                                            in1=psv2[:], op=OP.add)
                for m in range(5):
                    psT_ = transpose_to(aggnm[:, 320 + 32 * m:320 + 32 * m + 32],
                                        P, C2)
                    afm = poolu.tile([C2, P], BF16, tag="afm_t")
                    nc.scalar.copy(out=afm[:], in_=psT_[:C2, :P])
                    pst2 = psA.tile([C2, P], F32, tag="mmA", space="PSUM")
                    nc.tensor.matmul(pst2[:], lw["wo_t"][:], afm[:], start=True, stop=True)
                    nc.vector.tensor_tensor(out=upd_t[:, m, :], in0=tfm(m)[:, tsl],
                                            in1=pst2[:], op=OP.add)

            # transpose updated features to node-major
            snm = poolu.tile([P, C0], BF16, tag="snm")
            psn = transpose_to(upd_s[:], P, P)
            nc.scalar.copy(out=snm[:], in_=psn[:, :P])
            vnm = poolu.tile([P, C1, 3], BF16, tag="vnm")
            tnm = poolu.tile([P, C2, 5], BF16, tag="tnm")
            if last_v:
                for i in range(3):
                    psn = transpose_to(upd_v[:, i, :], C1, P)
                    nc.vector.tensor_copy(
                        out=reap(vnm[:, 0:1, i:i + 1], [(3, C1)]), in_=psn[:, :C1])
                for m in range(5):
                    psn = transpose_to(upd_t[:, m, :], C2, P)
                    nc.vector.tensor_copy(
                        out=reap(tnm[:, 0:1, m:m + 1], [(5, C2)]), in_=psn[:, :C2])

            # norm statistics first, then batched Ln / Exp (2 table loads)
            stats = poolu.tile([P, 6], F32, tag="stats")
            nc.vector.bn_stats(out=stats[:], in_=snm[:])
            mv = poolu.tile([P, 2], F32, tag="mv")
            nc.vector.bn_aggr(out=mv[:], in_=stats[:])
            lnt = poolu.tile([P, 2], F32, tag="lnt")
            vr2 = poolu.tile([P, 1], F32, tag="vr2")
            tr2 = poolu.tile([P, 1], F32, tag="tr2")
            if last_v:
                vsq = poolu.tile([P, C1, 3], BF16, tag="vsq")
                nc.vector.tensor_tensor(out=vsq[:], in0=vnm[:], in1=vnm[:], op=OP.mult)
                vr1 = poolu.tile([P, C1], F32, tag="vr1")
                nc.vector.tensor_reduce(out=vr1[:], in_=vsq[:], axis=mybir.AxisListType.X, op=OP.add)
                nc.vector.tensor_reduce(out=vr2[:], in_=vr1[:], axis=mybir.AxisListType.X, op=OP.add)
                tsq = poolu.tile([P, C2, 5], BF16, tag="tsq")
                nc.vector.tensor_tensor(out=tsq[:], in0=tnm[:], in1=tnm[:], op=OP.mult)
                tr1 = poolu.tile([P, C2], F32, tag="tr1")
                nc.vector.tensor_reduce(out=tr1[:], in_=tsq[:], axis=mybir.AxisListType.X, op=OP.add)
                nc.vector.tensor_reduce(out=tr2[:], in_=tr1[:], axis=mybir.AxisListType.X, op=OP.add)
            nc.scalar.activation(out=lnt[:, 0:1], in_=mv[:, 1:2], func=AF.Ln, bias=eps_t[:])
            if last_v:
                nc.scalar.activation(out=vr2[:], in_=vr2[:], func=AF.Ln, bias=eps_t[:], scale=1.0 / C1)
                nc.scalar.activation(out=tr2[:], in_=tr2[:], func=AF.Ln, bias=eps_t[:], scale=1.0 / C2)
            nc.scalar.activation(out=lnt[:, 1:2], in_=lnt[:, 0:1], func=AF.Exp, scale=-0.5)
            if last_v:
                nc.scalar.activation(out=vr2[:], in_=vr2[:], func=AF.Exp, scale=-0.5)
                nc.scalar.activation(out=tr2[:], in_=tr2[:], func=AF.Exp, scale=-0.5)
            # apply LayerNorm on s
            nc.vector.tensor_scalar(out=snm[:], in0=snm[:], scalar1=mv[:, 0:1],
                                    scalar2=lnt[:, 1:2], op0=OP.subtract, op1=OP.mult)
            nc.vector.tensor_tensor(out=snm[:], in0=snm[:], in1=lw["lngs"][:], op=OP.mult)
            nc.vector.tensor_tensor(out=snm[:], in0=snm[:], in1=lw["lnbs"][:], op=OP.add)
            if last_v:
                nc.vector.tensor_scalar(out=vnm[:], in0=vnm[:], scalar1=vr2[:],
                                        scalar2=None, op0=OP.mult)
                nc.vector.tensor_tensor(
                    out=vnm[:], in0=vnm[:],
                    in1=reap(lw["lngv"][:, 0:1], [(1, C1), (0, 3)]), op=OP.mult)
                nc.vector.tensor_scalar(out=tnm[:], in0=tnm[:], scalar1=tr2[:],
                                        scalar2=None, op0=OP.mult)
                nc.vector.tensor_tensor(
                    out=tnm[:], in0=tnm[:],
                    in1=reap(lw["lngt"][:, 0:1], [(1, C2), (0, 5)]), op=OP.mult)

            # write back feature-major stores
            psn = transpose_to(snm[:], P, P)
            nc.scalar.copy(out=sfm[:, tsl], in_=psn[:, :P])
            if last_v:
                for i in range(3):
                    psn = transpose_to(reap(vnm[:, 0:1, i:i + 1], [(3, C1)]), P, C1)
                    nc.scalar.copy(out=vfm(i)[:, tsl], in_=psn[:C1, :P])
                for m in range(5):
                    psn = transpose_to(reap(tnm[:, 0:1, m:m + 1], [(5, C2)]), P, C2)
                    nc.scalar.copy(out=tfm(m)[:, tsl], in_=psn[:C2, :P])

            if last_v:
                # next-layer node-table projections -> ntrow (node-major, bf16)
                ntrow = poolu.tile([P, GW], BF16, tag="ntrow")

                def proj_to_row(lhsT, rhs, rows, dims, off):
                    ps = psA.tile([P, P], F32, tag="mmA", space="PSUM")
                    nc.tensor.matmul(ps[:rows, :P], lhsT, rhs, start=True, stop=True)
                    sb = poolu.tile([P, P], BF16, tag="projsb")
                    nc.scalar.copy(out=sb[:rows, :P], in_=ps[:rows, :P])
                    psn2 = psA.tile([P, P], BF16, tag="mmA", space="PSUM")
                    nc.tensor.transpose(psn2[:P, :rows], sb[:rows, :P],
                                        ident_t[:rows, :rows])
                    nc.vector.tensor_copy(
                        out=reap(ntrow[:, off:off + 1], dims), in_=psn2[:P, :rows])

                proj_to_row(lw["ws_src2"][:], sfm[:, tsl], C0, [(BLK, 4), (1, 32)], O_S)
                proj_to_row(lw["ws_v2"][:], sfm[:, tsl], C1, [(BLK, 4), (1, 16)], O_SV)
                proj_to_row(lw["ws_t2"][:], sfm[:, tsl], C2, [(BLK, 4), (1, 8)], O_ST)
                for i in range(3):
                    proj_to_row(lw["wv_v2"][:], vfm(i)[:, tsl], C1,
                                [(BLK, 4), (1, 16)], O_V + 16 * i)
                for m in range(5):
                    proj_to_row(lw["wt_t2"][:], tfm(m)[:, tsl], C2,
                                [(BLK, 4), (1, 8)], O_T + 8 * m)
                nc.sync.dma_start(out=own[l][tsl, 0:GW], in_=ntrow[:])
            else:
                # final readout head for this tile
                feat = poolu.tile([P, 4, P], BF16, tag="feat")
                for b in range(4):
                    ps = psA.tile([P, P], F32, tag="mmA", space="PSUM")
                    nc.tensor.matmul(ps[:], wfeat_t[:, b * 128:(b + 1) * 128],
                                     sfm[:, tsl], start=True, stop=True)
                    nc.scalar.activation(out=feat[:, b, :], in_=ps[:],
                                         func=AF.Gelu_apprx_tanh, bias=bfeatp_t[:, b:b + 1])
                pse = psA.tile([1, P], F32, tag="mmA", space="PSUM")
                for b in range(4):
                    nc.tensor.matmul(pse[:], wout1p_t[:, b:b + 1], feat[:, b, :],
                                     start=(b == 0), stop=(b == 3))
                ne = poolu.tile([1, P], F32, tag="ne")
                nc.vector.tensor_copy(out=ne[:], in_=pse[:])
                nc.sync.dma_start(out=t_nodee[tsl], in_=ne[0:1, :])

        for l in range(LYR):
            gsrc = t_ntab0 if l == 0 else ntab[l - 1]
            lw = dict(
                wrad1=loadL(t_wrad1, l, NB, RAD, "wrad1", BF16),
                wrad2=loadL(t_wrad2, l, RAD, RAD, "wrad2", BF16),
                brad1=loadL(t_brad1, l, RAD, 1, "brad1"),
                brad2=loadL(t_brad2, l, RAD, 1, "brad2"),
                wwa=loadL(t_wwa, l, RAD, AW, "wwa", BF16),
                wwb=loadL(t_wwb, l, RAD, BW, "wwb", BF16),
                wo_s=loadL(t_wo_s, l, C0, C0, "wo_s", BF16),
                wo_v=loadL(t_wo_v, l, C1, C1, "wo_v", BF16),
                wo_t=loadL(t_wo_t, l, C2, C2, "wo_t", BF16),
                lngs=loadL(t_lngs, l, P, C0, "lngs", BF16),
                lnbs=loadL(t_lnbs, l, P, C0, "lnbs", BF16),
                lngv=loadL(t_lngv, l, P, C1, "lngv", BF16),
                lngt=loadL(t_lngt, l, P, C2, "lngt", BF16),
            )
            if l < LYR - 1:
                lw["ws_src2"] = loadL(t_ws_src, l + 1, C0, C0, "ws_src2", BF16)
                lw["ws_v2"] = loadL(t_ws_v, l + 1, C0, C1, "ws_v2", BF16)
                lw["ws_t2"] = loadL(t_ws_t, l + 1, C0, C2, "ws_t2", BF16)
                lw["wv_v2"] = loadL(t_wv_v, l + 1, C1, C1, "wv_v2", BF16)
                lw["wt_t2"] = loadL(t_wt_t, l + 1, C2, C2, "wt_t2", BF16)
            for t in range(ntile):
                em_s = poolT.tile([P, cpt * 4, EMW], F32, tag="em_s")
                nc.sync.dma_start(out=em_s[:], in_=t_em[:, t * cpt * 4:(t + 1) * cpt * 4, :])
                idx_s = poolT.tile([P, cpt * 4], mybir.dt.int32, tag="idx_s")
                nc.sync.dma_start(out=idx_s[:], in_=t_idxw[:, t * cpt * 4:(t + 1) * cpt * 4])
                agg = edge_tile(l, t, gsrc, lw, (em_s, idx_s, t * cpt))
                update_tile(l, t, agg, lw)
            if l < LYR - 1:
                nc.gpsimd.collective_compute(
                    "AllGather", OP.bypass,
                    replica_groups=[list(range(cfg.ncore))],
                    ins=[own[l][:]], outs=[ntab[l][:]])

    nc.compile()
    return nc


# ---------------- entry point ----------------

def _ensure_profile_hook():
    try:
        import antenv  # noqa
        import antenv.axon_hooks  # noqa
        return
    except Exception:
        pass
    try:
        import antenv
        from trn_agent_boot.trn_boot import _ntff_profile_via_ctypes
        hook = _ntff_profile_via_ctypes("/opt/axon/libaxon_pjrt.so")
        mod = types.ModuleType("antenv.axon_hooks")
        mod.get_axon_ntff_profile_hook = lambda: hook
        mod.set_axon_ntff_profile_hook = lambda h: None
        sys.modules["antenv.axon_hooks"] = mod
        antenv.axon_hooks = mod
    except Exception:
        pass


_PROGRAM_CACHE = {}


def run_cfg(inp, cfg: Cfg, trace=False):
    in_maps, pid_of = host_preprocess(inp, cfg)
    key = (cfg.ncore, cfg.npc, cfg.cpt, cfg.layers)
    if key not in _PROGRAM_CACHE:
        _PROGRAM_CACHE[key] = build_program(cfg)
    nc = _PROGRAM_CACHE[key]
    if trace:
        _ensure_profile_hook()
    res = run_bass_kernel_spmd(nc, in_maps, list(range(cfg.ncore)), trace=trace)
    node_e_p = np.concatenate(
        [res.results[c]["node_e"] for c in range(cfg.ncore)])
    node_e = node_e_p[pid_of]          # back to original node order
    return node_e, res


def kernel(**inputs):
    cfg = Cfg()
    node_e, _ = run_cfg(inputs, cfg)
    node_e = node_e[:, None] + np.asarray(inputs["b_out1"], np.float32)[None, :]
    batch = np.asarray(inputs["batch"]).astype(np.int64)
    graph = np.zeros((G, 1), np.float32)
    np.add.at(graph, batch, node_e)
    out = graph @ np.asarray(inputs["W_read"], np.float32) + np.asarray(
        inputs["b_read"], np.float32)
    return out.astype(np.float32)


# revision 27
# speedup vs baseline: 1.2258x; 1.2258x over previous
"""Trainium2 Bass kernel for the Equiformer-style GNN regressor.

Strategy (8 NeuronCores, SPMD), v2 — bf16 rewrite of the fp32 baseline:
  - Nodes are permuted on host (LPT on in-degree) so each of the 80
    node-tiles (8 cores x 10 tiles of 128) owns 125 real nodes and at most
    4096 incoming edges -> cpt=8 chunks of 512 edge slots per tile.
  - Per layer, a DRAM node table ([10240, 640] bf16, 4 head-blocks of 144:
    [s 32 | v 48 | t 40 | sv 16 | st 8] + pad) holds per-node projected
    quantities. Edge phase gathers rows edge-major, computes radial
    projections DIRECTLY edge-major on PE (lhsT = radial-MLP weights chunk,
    rhs = packed per-layer wwall) into PSUM -- no transposes -- and forms
    messages with DVE ops reading PSUM. Aggregation per dst node-tile is a
    single pair of one-hot bf16 matmuls per 128-edge sub (den + 480-wide
    strided message rhs) accumulating into one PSUM bank.
  - RBF basis (layer-independent) is computed once in fp32 (numerically
    sensitive quadratic form) and kept SBUF-resident in bf16.
  - Update phase (fp32): out-projections + residual + equivariant norms +
    next-layer projections; new bf16 table rows AllGathered across cores.
  - Final readout (per-node energies) is DMA'd out; per-graph segment-sum +
    Linear(1,1) runs on host.
"""
import math
import sys
import types
from contextlib import ExitStack
from dataclasses import dataclass

import numpy as np
import ml_dtypes

import concourse.bacc as bacc
import concourse.bass as bass
import concourse.tile as tile
from concourse import mybir
from concourse.bass_utils import run_bass_kernel_spmd

F32 = mybir.dt.float32
BF16 = mybir.dt.bfloat16
AF = mybir.ActivationFunctionType
OP = mybir.AluOpType
BF = ml_dtypes.bfloat16

# ---------------- problem constants (hardcoded per spec) ----------------
N, E, G, L = 10000, 320000, 32, 6
C0, C1, C2, H, NB, RAD, FD, T = 128, 64, 32, 4, 128, 64, 512, 1
MAXR = 5.0
EPS = 1e-6
NCORE = 8
P = 128

BLK = 144         # per-head block width in table rows and rp banks
GW = 4 * BLK      # used table row width (576)
GWP = 640         # padded table row width
AW = 512             # rp bank A: [R_sa 0:128 | b0 128:272 | b1 320:464]
BW = 336             # rp bank B: [b2 0:144 | b3 192:336]
RPS = 192            # uniform head-block stride inside the rp psum tile
# offsets inside a head block
O_S, O_V, O_T, O_SV, O_ST = 0, 32, 80, 120, 136
HB = 148          # hh per-head block: [ex 1 | 144 msg block | pad 3]
HHW = 4 * HB      # 592

EM_DST, EM_MASK, EM_SH1, EM_SH2, EMW = 0, 1, 2, 5, 12


@dataclass
class Cfg:
    ncore: int = NCORE
    npc: int = 1280          # padded nodes per core (multiple of 128)
    cpt: int = 8             # chunks (512 slots) per node-tile
    layers: int = L
    nn: int = N              # real node count

    @property
    def ntile(self):
        return self.npc // P

    @property
    def np_total(self):
        return self.npc * self.ncore

    @property
    def slots(self):
        return self.ntile * self.cpt * 512


# ---------------- host-side packing helpers ----------------

def g_col_maps():
    """Column index maps for the 640-wide node-table row layout."""
    ps = np.zeros(C0, np.int64)
    psv = np.zeros(C1, np.int64)
    pst = np.zeros(C2, np.int64)
    pv = np.zeros((C1, 3), np.int64)
    pt = np.zeros((C2, 5), np.int64)
    for c in range(C0):
        h, j = divmod(c, 32)
        ps[c] = BLK * h + O_S + j
    for c in range(C1):
        h, j = divmod(c, 16)
        psv[c] = BLK * h + O_SV + j
        for i in range(3):
            pv[c, i] = BLK * h + O_V + 16 * i + j
    for c in range(C2):
        h, j = divmod(c, 8)
        pst[c] = BLK * h + O_ST + j
        for m in range(5):
            pt[c, m] = BLK * h + O_T + 8 * m + j
    return ps, psv, pst, pv, pt


G_PS, G_PSV, G_PST, G_PV, G_PT = g_col_maps()


def pack_node_table(P_s, P_sv, P_st, Pv, Pt):
    """[n,C0],[n,C1],[n,C2],[n,C1,3],[n,C2,5] -> [n, 640] bf16."""
    n = P_s.shape[0]
    out = np.zeros((n, GWP), np.float32)
    out[:, G_PS] = P_s
    out[:, G_PSV] = P_sv
    out[:, G_PST] = P_st
    out[:, G_PV.reshape(-1)] = Pv.reshape(n, -1)
    out[:, G_PT.reshape(-1)] = Pt.reshape(n, -1)
    return out.astype(BF)


def pack_wwall(Ww_s, Ww_v, Ww_t, Ww_vv, Ww_tt, attn_a):
    """Per-layer radial-projection weights -> bankA [RAD,512], bankB [RAD,336].
    Head blocks sit at uniform stride RPS=192 from col 128 of the combined
    [P,1024] rp psum tile (bank B lands at tile col 512)."""
    full = np.zeros((RAD, 1024), np.float32)
    for c in range(C0):
        h, j = divmod(c, 32)
        full[:, c] = Ww_s[:, c] * attn_a[h, j]
    for h in range(H):
        base = 128 + RPS * h
        full[:, base + O_S:base + O_S + 32] = Ww_s[:, 32 * h:32 * h + 32]
        for c in range(16):
            for i in range(3):
                full[:, base + O_V + 16 * i + c] = Ww_vv[:, 16 * h + c]
        for c in range(8):
            for m in range(5):
                full[:, base + O_T + 8 * m + c] = Ww_tt[:, 8 * h + c]
        full[:, base + O_SV:base + O_SV + 16] = Ww_v[:, 16 * h:16 * h + 16]
        full[:, base + O_ST:base + O_ST + 8] = Ww_t[:, 8 * h:8 * h + 8]
    return full[:, :AW].astype(BF), full[:, 512:512 + BW].astype(BF)


def balance_nodes(edst, cfg: Cfg):
    """LPT-assign nodes to the 80 (core,tile) bins, 125 real nodes each.
    Returns pid_of[node] (padded id) and the inverse slot list."""
    nbin = cfg.ncore * cfg.ntile
    per_bin = cfg.nn // nbin
    assert per_bin * nbin == cfg.nn
    deg = np.bincount(edst, minlength=cfg.nn)
    order = np.argsort(-deg, kind="stable")
    bin_edges = np.zeros(nbin, np.int64)
    bin_count = np.zeros(nbin, np.int64)
    pid_of = np.zeros(cfg.nn, np.int64)
    import heapq
    heap = [(0, b) for b in range(nbin)]
    heapq.heapify(heap)
    for nd in order:
        while True:
            e, b = heapq.heappop(heap)
            if bin_count[b] < per_bin:
                break
        c, t = divmod(b, cfg.ntile)
        pid_of[nd] = c * cfg.npc + t * P + bin_count[b]
        bin_count[b] += 1
        bin_edges[b] += deg[nd]
        if bin_count[b] < per_bin:
            heapq.heappush(heap, (bin_edges[b], b))
    assert bin_edges.max() <= cfg.cpt * 512, f"tile overflow {bin_edges.max()}"
    return pid_of


def host_preprocess(inp, cfg: Cfg):
    """Build all per-core device input arrays + schedule."""
    npc, ncore, ntile = cfg.npc, cfg.ncore, cfg.ntile

    pos = np.asarray(inp["pos"], np.float32)
    node_atom = np.asarray(inp["node_atom"]).astype(np.int64)
    esrc = np.asarray(inp["edge_src"]).astype(np.int64)
    edst = np.asarray(inp["edge_dst"]).astype(np.int64)

    pid_of = balance_nodes(edst, cfg)
    src_p = pid_of[esrc]
    dst_p = pid_of[edst]

    # geometry (match reference formulas, f32)
    rel = pos[edst] - pos[esrc]
    d2 = (rel * rel).sum(-1) + np.float32(EPS)
    d = np.sqrt(d2)
    u = rel / d[:, None]
    s3, s5, s15 = [np.float32(np.sqrt(x)) for x in (3.0, 5.0, 15.0)]
    sh1 = s3 * u
    x_, y_, z_ = u[:, 0], u[:, 1], u[:, 2]
    sh2 = np.stack(
        [s15 * x_ * y_, s15 * y_ * z_, np.float32(0.5) * s5 * (3 * z_ * z_ - 1.0),
         s15 * x_ * z_, np.float32(0.5) * s15 * (x_ * x_ - y_ * y_)], -1)

    # per-(core,tile) edge lists
    tile_of_edge = (dst_p // npc) * ntile + (dst_p % npc) // P
    order = np.argsort(tile_of_edge, kind="stable")
    counts = np.bincount(tile_of_edge, minlength=ncore * ntile)
    assert counts.max() <= cfg.cpt * 512
    starts = np.zeros(ncore * ntile + 1, np.int64)
    np.cumsum(counts, out=starts[1:])

    S = cfg.slots
    per_core = []
    for c in range(ncore):
        slot_src = np.zeros(S, np.int64)
        slot_dstrel = np.zeros(S, np.float32)
        slot_mask = np.zeros(S, np.float32)
        slot_d2 = np.ones(S, np.float32)
        slot_d = np.ones(S, np.float32)
        slot_sh1 = np.zeros((S, 3), np.float32)
        slot_sh2 = np.zeros((S, 5), np.float32)
        for t in range(ntile):
            gt = c * ntile + t
            eids = order[starts[gt]:starts[gt + 1]]
            base = t * cfg.cpt * 512
            k = len(eids)
            sl = slice(base, base + k)
            slot_src[sl] = src_p[eids]
            slot_dstrel[sl] = (dst_p[eids] - (c * npc + t * P)).astype(np.float32)
            slot_mask[sl] = 1.0
            slot_d2[sl] = d2[eids]
            slot_d[sl] = d[eids]
            slot_sh1[sl] = sh1[eids]
            slot_sh2[sl] = sh2[eids]

        nsub = S // P
        em = np.zeros((P, nsub, EMW), np.float32)
        j = np.arange(S)
        em[j % P, j // P, EM_DST] = slot_dstrel
        em[j % P, j // P, EM_MASK] = slot_mask
        em[j % P, j // P, EM_SH1:EM_SH1 + 3] = slot_sh1
        em[j % P, j // P, EM_SH2:EM_SH2 + 5] = slot_sh2
        geom = np.ones((3, S), np.float32)
        geom[0] = slot_d2
        geom[1] = slot_d
        idx_em = np.zeros((P, nsub), np.int32)
        idx_em[j % P, j // P] = slot_src.astype(np.int32)
        per_core.append(dict(em=em, geom=geom, idxw=idx_em))

    # ---- weights ----
    wd = {}
    centers = np.linspace(0.0, MAXR, NB).astype(np.float32)
    width = np.float32(MAXR / NB)
    wq = np.zeros((3, NB), np.float32)
    wq[0] = -0.5 / width**2
    wq[1] = centers / width**2
    wq[2] = -0.5 * centers**2 / width**2
    wd["wq"] = wq
    wd["wrad1"] = np.asarray(inp["Wrad1"], np.float32).astype(BF)
    wd["wrad2"] = np.asarray(inp["Wrad2"], np.float32).astype(BF)
    wd["brad1"] = np.asarray(inp["brad1"], np.float32).reshape(cfg.layers, RAD, 1)
    wd["brad2"] = np.asarray(inp["brad2"], np.float32).reshape(cfg.layers, RAD, 1)
    wwA, wwB = [], []
    for l in range(cfg.layers):
        a, b = pack_wwall(inp["Ww_s"][l], inp["Ww_v"][l], inp["Ww_t"][l],
                          inp["Ww_vv"][l], inp["Ww_tt"][l], inp["attn_a"][l])
        wwA.append(a)
        wwB.append(b)
    wd["wwa"] = np.stack(wwA)
    wd["wwb"] = np.stack(wwB)
    wd["wo_s"] = np.asarray(inp["Wo_s"], np.float32).astype(BF)
    wd["wo_v"] = np.asarray(inp["Wo_v"], np.float32).astype(BF)
    wd["wo_t"] = np.asarray(inp["Wo_t"], np.float32).astype(BF)
    wd["ws_src"] = np.asarray(inp["Ws_src"], np.float32).astype(BF)
    wd["ws_v"] = np.asarray(inp["Ws_v"], np.float32).astype(BF)
    wd["ws_t"] = np.asarray(inp["Ws_t"], np.float32).astype(BF)
    wd["wv_v"] = np.asarray(inp["Wv_v"], np.float32).astype(BF)
    wd["wt_t"] = np.asarray(inp["Wt_t"], np.float32).astype(BF)
    rep = lambda a: np.broadcast_to(a[:, None, :], (a.shape[0], P, a.shape[1])).copy()
    wd["lngs"] = rep(np.asarray(inp["g_s"], np.float32)).astype(BF)
    wd["lnbs"] = rep(np.asarray(inp["b_s"], np.float32)).astype(BF)
    wd["lngv"] = rep(np.asarray(inp["g_v"], np.float32)).astype(BF)
    wd["lngt"] = rep(np.asarray(inp["g_t"], np.float32)).astype(BF)
    wd["wfeat"] = np.asarray(inp["W_feat"], np.float32).astype(BF)
    wd["bfeatp"] = np.asarray(inp["b_feat"], np.float32).reshape(4, 128).T.copy()
    wd["wout1p"] = np.asarray(inp["W_out1"], np.float32).reshape(4, 128).T.copy().astype(BF)
    wd["nidx"] = np.tile(np.arange(P, dtype=np.float32), (P, 1)).astype(BF)
    wd["identb"] = np.eye(P, dtype=np.float32).astype(BF)

    # ---- initial node table (layer 0 projections) + s0 feature-major ----
    s0 = np.asarray(inp["atom_emb"], np.float32)[node_atom]     # [N, C0]
    s0p = np.zeros((cfg.np_total, C0), np.float32)
    s0p[pid_of] = s0
    ntab0 = pack_node_table(
        s0p @ inp["Ws_src"][0], s0p @ inp["Ws_v"][0], s0p @ inp["Ws_t"][0],
        np.zeros((cfg.np_total, C1, 3), np.float32),
        np.zeros((cfg.np_total, C2, 5), np.float32))

    in_maps = []
    for c in range(ncore):
        m = dict(per_core[c])
        m["ntab0"] = ntab0
        m["s0fm"] = s0p[c * npc:(c + 1) * npc].T.copy().astype(BF)   # [C0, npc]
        for k, v in wd.items():
            m[k] = v
        in_maps.append(m)
    return in_maps, pid_of


# ---------------- device program ----------------

def reap(sliced: bass.AP, dims) -> bass.AP:
    """Rebuild the free-dims of a (narrow) sliced AP with explicit
    [step, count] pairs, keeping its partition dim and offset."""
    return bass.AP(sliced.tensor, sliced.offset,
                   [list(sliced.ap[0])] + [[int(s), int(c)] for s, c in dims])


def build_program(cfg: Cfg):
    nc = bacc.Bacc("TRN2", target_bir_lowering=False, debug=False,
                   enable_asserts=True, num_devices=cfg.ncore)
    npc, ntile, cpt = cfg.npc, cfg.ntile, cfg.cpt
    S = cfg.slots
    nsub = S // P
    NPT = cfg.np_total
    LYR = cfg.layers

    dp = nc.declare_dram_parameter
    t_ntab0 = dp("ntab0", [NPT, GWP], BF16, isOutput=False)
    t_s0fm = dp("s0fm", [C0, npc], BF16, isOutput=False)
    t_em = dp("em", [P, nsub, EMW], F32, isOutput=False)
    t_geom = dp("geom", [3, S], F32, isOutput=False)
    t_idxw = dp("idxw", [P, nsub], mybir.dt.int32, isOutput=False)
    t_wq = dp("wq", [3, NB], F32, isOutput=False)
    t_wrad1 = dp("wrad1", [LYR, NB, RAD], BF16, isOutput=False)
    t_wrad2 = dp("wrad2", [LYR, RAD, RAD], BF16, isOutput=False)
    t_brad1 = dp("brad1", [LYR, RAD, 1], F32, isOutput=False)
    t_brad2 = dp("brad2", [LYR, RAD, 1], F32, isOutput=False)
    t_wwa = dp("wwa", [LYR, RAD, AW], BF16, isOutput=False)
    t_wwb = dp("wwb", [LYR, RAD, BW], BF16, isOutput=False)
    t_wo_s = dp("wo_s", [LYR, C0, C0], BF16, isOutput=False)
    t_wo_v = dp("wo_v", [LYR, C1, C1], BF16, isOutput=False)
    t_wo_t = dp("wo_t", [LYR, C2, C2], BF16, isOutput=False)
    t_ws_src = dp("ws_src", [LYR, C0, C0], BF16, isOutput=False)
    t_ws_v = dp("ws_v", [LYR, C0, C1], BF16, isOutput=False)
    t_ws_t = dp("ws_t", [LYR, C0, C2], BF16, isOutput=False)
    t_wv_v = dp("wv_v", [LYR, C1, C1], BF16, isOutput=False)
    t_wt_t = dp("wt_t", [LYR, C2, C2], BF16, isOutput=False)
    t_lngs = dp("lngs", [LYR, P, C0], BF16, isOutput=False)
    t_lnbs = dp("lnbs", [LYR, P, C0], BF16, isOutput=False)
    t_lngv = dp("lngv", [LYR, P, C1], BF16, isOutput=False)
    t_lngt = dp("lngt", [LYR, P, C2], BF16, isOutput=False)
    t_wfeat = dp("wfeat", [C0, FD], BF16, isOutput=False)
    t_bfeatp = dp("bfeatp", [P, 4], F32, isOutput=False)
    t_wout1p = dp("wout1p", [P, 4], BF16, isOutput=False)
    t_nidx = dp("nidx", [P, P], BF16, isOutput=False)
    t_identb = dp("identb", [P, P], BF16, isOutput=False)
    t_nodee = dp("node_e", [npc], F32, isOutput=True)

    own = [nc.dram_tensor(f"own{l}", [npc, GWP], BF16) for l in range(LYR - 1)]
    ntab = [nc.dram_tensor(f"ntab{l + 1}", [NPT, GWP], BF16, addr_space="Shared")
            for l in range(LYR - 1)]

    with tile.TileContext(nc) as tc, ExitStack() as ctx:
        pool1 = ctx.enter_context(tc.tile_pool(name="const", bufs=1))
        poolL = ctx.enter_context(tc.tile_pool(name="layerw", bufs=1))
        poolT = ctx.enter_context(tc.tile_pool(name="tilec", bufs=2))
        poolg = ctx.enter_context(tc.tile_pool(name="gath", bufs=2))
        poole = ctx.enter_context(tc.tile_pool(name="edge", bufs=2))
        poolx = ctx.enter_context(tc.tile_pool(name="edge1", bufs=2))
        poolu = ctx.enter_context(tc.tile_pool(name="upd", bufs=1))
        psA = ctx.enter_context(tc.tile_pool(name="psA", bufs=2, space="PSUM"))
        psRP = ctx.enter_context(tc.tile_pool(name="psRP", bufs=2, space="PSUM"))
        psAgg = ctx.enter_context(tc.tile_pool(name="psAgg", bufs=1, space="PSUM"))

        def load1(dram, shape, dtype=F32):
            t = pool1.tile(shape, dtype, tag=dram.name)
            nc.sync.dma_start(out=t[:], in_=dram[:])
            return t

        # resident constants
        wq_t = load1(t_wq, [3, NB])
        nidx_t = load1(t_nidx, [P, P], BF16)
        ident_t = load1(t_identb, [P, P], BF16)
        wfeat_t = load1(t_wfeat, [C0, FD], BF16)
        bfeatp_t = load1(t_bfeatp, [P, 4])
        wout1p_t = load1(t_wout1p, [P, 4], BF16)

        eps_t = pool1.tile([P, 1], F32, tag="epsT")
        nc.vector.memset(eps_t[:], EPS)

        # feature-major stores for own nodes (bf16)
        sfm = pool1.tile([C0, npc], BF16, tag="sfm")
        nc.sync.dma_start(out=sfm[:], in_=t_s0fm[:])
        vfm_t = pool1.tile([C1, 3, npc], BF16, tag="vfm")
        nc.vector.memset(vfm_t[:], 0.0)
        tfm_t = pool1.tile([C2, 5, npc], BF16, tag="tfm")
        nc.vector.memset(tfm_t[:], 0.0)

        def vfm(i):
            return vfm_t[:, i, :]

        def tfm(m):
            return tfm_t[:, m, :]

        # ---- RBF pre-pass: layer-independent, fp32 quadratic form ----
        rbf_all = pool1.tile([NB, S], BF16, tag="rbf_all")
        for k in range(S // 512):
            gsl = poolT.tile([3, 512], F32, tag="geom_c")
            nc.sync.dma_start(out=gsl[:], in_=t_geom[:, k * 512:(k + 1) * 512])
            ps = psA.tile([NB, 512], F32, tag="mmA", space="PSUM")
            nc.tensor.matmul(ps[:], wq_t[:], gsl[:], start=True, stop=True)
            nc.scalar.activation(out=rbf_all[:, k * 512:(k + 1) * 512],
                                 in_=ps[:], func=AF.Exp)

        def loadL(dram, l, p, f, tag, dtype=F32):
            t = poolL.tile([p, f], dtype, tag=tag)
            nc.sync.dma_start(out=t[:], in_=dram[l])
            return t

        def edge_tile(l, t, gsrc, lw, tct):
            """Edge phase for node-tile t of layer l. Returns agg psum tile.

            agg layout: 4 head blocks of AGB: [den 1 | s 32 (| v 48 | t 40)].
            """
            em_s, idx_s, tbase = tct
            last_v = l < LYR - 1
            wlen = BLK if last_v else 32     # msg cols per head block
            agb = 120 if last_v else 32      # aggregated msg cols per block
            agg = psAgg.tile([P, 484], F32, tag="agg", space="PSUM")
            # phase 1: radial MLP for all chunks (batched silus, one table load)
            w_b = poolx.tile([RAD, cpt * 512], BF16, tag="w_b")
            for k in range(cpt):
                sbase = (tbase + k) * 512
                ps1 = psA.tile([RAD, 512], F32, tag="mmA", space="PSUM")
                nc.tensor.matmul(ps1[:], lw["wrad1"][:],
                                 rbf_all[:, sbase:sbase + 512], start=True, stop=True)
                h1 = poolx.tile([RAD, 512], BF16, tag="h1")
                nc.scalar.activation(out=h1[:], in_=ps1[:], func=AF.Silu,
                                     bias=lw["brad1"][:])
                ps2 = psA.tile([RAD, 512], F32, tag="mmA", space="PSUM")
                nc.tensor.matmul(ps2[:], lw["wrad2"][:], h1[:], start=True, stop=True)
                nc.scalar.activation(out=w_b[:, k * 512:(k + 1) * 512], in_=ps2[:],
                                     func=AF.Silu, bias=lw["brad2"][:])
            # phase 2: gather + messages + aggregation
            for k in range(cpt):
                gt = poolg.tile([P, 4, GWP], BF16, tag="gt")
                for sub4 in range(4):
                    nc.gpsimd.indirect_dma_start(
                        out=gt[:, sub4, :], out_offset=None, in_=gsrc[:, :],
                        in_offset=bass.IndirectOffsetOnAxis(
                            ap=idx_s[:, k * 4 + sub4:k * 4 + sub4 + 1], axis=0))
                logit = poole.tile([P, 16], F32, tag="logit")
                ex = poole.tile([P, 16], BF16, tag="ex")
                hh = poole.tile([P, 4, GW], BF16, tag="hh")
                scr = poole.tile([P, 128], F32, tag="scr")
                t2vt = poolx.tile([P, 4, 88], BF16, tag="t2vt")
                for sub in range(4):
                    cs = k * 4 + sub
                    wsl = w_b[:, (k * 4 + sub) * 128:(k * 4 + sub + 1) * 128]
                    # radial projections, direct edge-major, one 2-bank tile:
                    # head blocks at uniform stride RPS from col 128
                    rp = psRP.tile([P, 1024], F32, tag="rp", space="PSUM")
                    nc.tensor.matmul(rp[:, 0:AW], wsl, lw["wwa"][:],
                                     start=True, stop=True)
                    if last_v:
                        nc.tensor.matmul(rp[:, 512:512 + BW], wsl, lw["wwb"][:],
                                         start=True, stop=True)
                    else:
                        # last layer: only R_s of head blocks 2,3
                        nc.tensor.matmul(rp[:, 512:512 + BW], wsl, lw["wwb"][:],
                                         start=True, stop=True)
                    # logits: (G_s * R_sa) summed over the 32 head channels
                    nc.vector.tensor_tensor(
                        out=scr[:],
                        in0=reap(gt[:, sub:sub + 1, 0:1], [(BLK, 4), (1, 32)]),
                        in1=rp[:, 0:128],
                        op=OP.mult)
                    nc.vector.tensor_reduce(
                        out=logit[:, sub * 4:sub * 4 + 4],
                        in_=scr[:].rearrange("p (h c) -> p h c", h=H),
                        axis=mybir.AxisListType.X, op=OP.add)
                    exs = slice(sub * 4, sub * 4 + 4)
                    # no logit mask needed: pad rows gather node 0, so logits
                    # stay O(1) and exp() is finite; the one-hot mask kills
                    # every pad contribution (den and messages alike)
                    nc.scalar.activation(out=ex[:, exs], in_=logit[:, exs],
                                         func=AF.Exp)
                    # hh msg cols = G * rp  (one TT across all 4 head blocks)
                    nc.vector.tensor_tensor(
                        out=reap(hh[:, sub:sub + 1, 0:1], [(BLK, 4), (1, wlen)]),
                        in0=reap(gt[:, sub:sub + 1, 0:1], [(BLK, 4), (1, wlen)]),
                        in1=reap(rp[:, 128:129], [(RPS, 4), (1, wlen)]),
                        op=OP.mult)
                    # hh msg *= alpha (broadcast per head block)
                    nc.vector.tensor_tensor(
                        out=reap(hh[:, sub:sub + 1, 0:1], [(BLK, 4), (1, wlen)]),
                        in0=reap(hh[:, sub:sub + 1, 0:1], [(BLK, 4), (1, wlen)]),
                        in1=reap(ex[:, sub * 4:sub * 4 + 1], [(1, 4), (0, wlen)]),
                        op=OP.mult)
                    if last_v:
                        sh1a = em_s[:, cs:cs + 1, EM_SH1:EM_SH1 + 1]
                        sh2a = em_s[:, cs:cs + 1, EM_SH2:EM_SH2 + 1]
                        # T2v/T2t = (alpha*A_v/t) outer sh, into one tile
                        # laid out like the hh [v 48 | t 40] block interior
                        nc.vector.tensor_tensor(
                            out=reap(t2vt[:, 0:1, 0:1], [(88, 4), (16, 3), (1, 16)]),
                            in0=reap(hh[:, sub:sub + 1, O_SV:O_SV + 1],
                                     [(BLK, 4), (0, 3), (1, 16)]),
                            in1=reap(sh1a, [(0, 4), (1, 3), (0, 16)]),
                            op=OP.mult)
                        nc.vector.tensor_tensor(
                            out=reap(t2vt[:, 0:1, 48:49], [(88, 4), (8, 5), (1, 8)]),
                            in0=reap(hh[:, sub:sub + 1, O_ST:O_ST + 1],
                                     [(BLK, 4), (0, 5), (1, 8)]),
                            in1=reap(sh2a, [(0, 4), (1, 5), (0, 8)]),
                            op=OP.mult)
                        # msg_v += T2v and msg_t += T2t in ONE add
                        nc.vector.tensor_tensor(
                            out=reap(hh[:, sub:sub + 1, O_V:O_V + 1],
                                     [(BLK, 4), (1, 88)]),
                            in0=reap(t2vt[:, 0:1, 0:1], [(88, 4), (1, 88)]),
                            in1=reap(hh[:, sub:sub + 1, O_V:O_V + 1],
                                     [(BLK, 4), (1, 88)]),
                            op=OP.add)
                    # one-hot with the pad mask folded in, then ONE agg matmul
                    first = (k == 0 and sub == 0)
                    last = (k == cpt - 1 and sub == 3)
                    msk = em_s[:, cs, EM_MASK:EM_MASK + 1]
                    oh = poole.tile([P, P], BF16, tag="oh")
                    nc.vector.tensor_scalar(out=oh[:], in0=nidx_t[:],
                                            scalar1=em_s[:, cs, EM_DST:EM_DST + 1],
                                            scalar2=msk, op0=OP.is_equal, op1=OP.mult)
                    nc.tensor.matmul(agg[:, 480:484], oh[:], ex[:, exs],
                                     start=first, stop=last, skip_group_check=True)
                    nc.tensor.matmul(
                        agg[:, 0:4 * agb], oh[:],
                        reap(hh[:, sub:sub + 1, 0:1], [(BLK, 4), (1, agb)]),
                        start=False, stop=last, skip_group_check=True)
            return agg

        def transpose_to(src_ap, kparts, ffree):
            """transpose src [kparts, ffree] sbuf bf16 -> psum bf16 [ffree, kparts]"""
            ps = psA.tile([P, P], BF16, tag="mmA", space="PSUM")
            nc.tensor.transpose(ps[:ffree, :kparts], src_ap,
                                ident_t[:kparts, :kparts])
            return ps

        def update_tile(l, t, agg, lw):
            tsl = slice(t * P, (t + 1) * P)
            last_v = l < LYR - 1
            agb = 120 if last_v else 32
            rden = poolu.tile([P, H], F32, tag="rden")
            nc.vector.tensor_scalar(out=rden[:], in0=agg[:, 480:484],
                                    scalar1=1e-9, scalar2=None, op0=OP.add)
            nc.vector.reciprocal(out=rden[:], in_=rden[:])
            # component-major aggnm (bf16): s 0:128 (32h+j), v 128+64i+16h+j,
            # t 320+32m+8h+j -- so the transposes below read contiguous cols
            aggnm = poolu.tile([P, 480], BF16, tag="aggnm")
            for h in range(H):
                nc.vector.tensor_scalar(
                    out=aggnm[:, 32 * h:32 * h + 32],
                    in0=agg[:, agb * h:agb * h + 32],
                    scalar1=rden[:, h:h + 1], scalar2=None, op0=OP.mult)
                if last_v:
                    nc.vector.tensor_scalar(
                        out=reap(aggnm[:, 128 + 16 * h:128 + 16 * h + 1],
                                 [(64, 3), (1, 16)]),
                        in0=reap(agg[:, agb * h + O_V:agb * h + O_V + 1],
                                 [(16, 3), (1, 16)]),
                        scalar1=rden[:, h:h + 1], scalar2=None, op0=OP.mult)
                    nc.vector.tensor_scalar(
                        out=reap(aggnm[:, 320 + 8 * h:320 + 8 * h + 1],
                                 [(32, 5), (1, 8)]),
                        in0=reap(agg[:, agb * h + O_T:agb * h + O_T + 1],
                                 [(8, 5), (1, 8)]),
                        scalar1=rden[:, h:h + 1], scalar2=None, op0=OP.mult)

            # transpose agg to feature-major + out-projections + residual
            psS = transpose_to(aggnm[:, 0:128], P, P)
            afm_s = poolu.tile([P, P], BF16, tag="afm_s")
            nc.scalar.copy(out=afm_s[:], in_=psS[:, :P])
            pso = psA.tile([P, P], F32, tag="mmA", space="PSUM")
            nc.tensor.matmul(pso[:], lw["wo_s"][:], afm_s[:], start=True, stop=True)
            upd_s = poolu.tile([P, P], BF16, tag="upd_s")
            nc.vector.tensor_tensor(out=upd_s[:], in0=sfm[:, tsl], in1=pso[:], op=OP.add)

            upd_v = poolu.tile([C1, 3, P], BF16, tag="upd_v")
            upd_t = poolu.tile([C2, 5, P], BF16, tag="upd_t")
            if last_v:
                for i in range(3):
                    psV = transpose_to(aggnm[:, 128 + 64 * i:128 + 64 * i + 64],
                                       P, C1)
                    afm = poolu.tile([C1, P], BF16, tag="afm_v")
                    nc.scalar.copy(out=afm[:], in_=psV[:C1, :P])
                    psv2 = psA.tile([C1, P], F32, tag="mmA", space="PSUM")
                    nc.tensor.matmul(psv2[:], lw["wo_v"][:], afm[:], start=True, stop=True)
                    nc.vector.tensor_tensor(out=upd_v[:, i, :], in0=vfm(i)[:, tsl],
                                            in1=psv2[:], op=OP.add)
                for m in range(5):
                    psT_ = transpose_to(aggnm[:, 320 + 32 * m:320 + 32 * m + 32],
                                        P, C2)
                    afm = poolu.tile([C2, P], BF16, tag="afm_t")
                    nc.scalar.copy(out=afm[:], in_=psT_[:C2, :P])
                    pst2 = psA.tile([C2, P], F32, tag="mmA", space="PSUM")
                    nc.tensor.matmul(pst2[:], lw["wo_t"][:], afm[:], start=True, stop=True)
                    nc.vector.tensor_tensor(out=upd_t[:, m, :], in0=tfm(m)[:, tsl],
                                            in1=pst2[:], op=OP.add)

            # transpose updated features to node-major
            snm = poolu.tile([P, C0], BF16, tag="snm")
            psn = transpose_to(upd_s[:], P, P)
            nc.scalar.copy(out=snm[:], in_=psn[:, :P])
            vnm = poolu.tile([P, C1, 3], BF16, tag="vnm")
            tnm = poolu.tile([P, C2, 5], BF16, tag="tnm")
            if last_v:
                for i in range(3):
                    psn = transpose_to(upd_v[:, i, :], C1, P)
                    nc.vector.tensor_copy(
                        out=reap(vnm[:, 0:1, i:i + 1], [(3, C1)]), in_=psn[:, :C1])
                for m in range(5):
                    psn = transpose_to(upd_t[:, m, :], C2, P)
                    nc.vector.tensor_copy(
                        out=reap(tnm[:, 0:1, m:m + 1], [(5, C2)]), in_=psn[:, :C2])

            # norm statistics first, then batched Ln / Exp (2 table loads)
            stats = poolu.tile([P, 6], F32, tag="stats")
            nc.vector.bn_stats(out=stats[:], in_=snm[:])
            mv = poolu.tile([P, 2], F32, tag="mv")
            nc.vector.bn_aggr(out=mv[:], in_=stats[:])
            lnt = poolu.tile([P, 2], F32, tag="lnt")
            vr2 = poolu.tile([P, 1], F32, tag="vr2")
            tr2 = poolu.tile([P, 1], F32, tag="tr2")
            if last_v:
                vsq = poolu.tile([P, C1, 3], BF16, tag="vsq")
                nc.vector.tensor_tensor(out=vsq[:], in0=vnm[:], in1=vnm[:], op=OP.mult)
                vr1 = poolu.tile([P, C1], F32, tag="vr1")
                nc.vector.tensor_reduce(out=vr1[:], in_=vsq[:], axis=mybir.AxisListType.X, op=OP.add)
                nc.vector.tensor_reduce(out=vr2[:], in_=vr1[:], axis=mybir.AxisListType.X, op=OP.add)
                tsq = poolu.tile([P, C2, 5], BF16, tag="tsq")
                nc.vector.tensor_tensor(out=tsq[:], in0=tnm[:], in1=tnm[:], op=OP.mult)
                tr1 = poolu.tile([P, C2], F32, tag="tr1")
                nc.vector.tensor_reduce(out=tr1[:], in_=tsq[:], axis=mybir.AxisListType.X, op=OP.add)
                nc.vector.tensor_reduce(out=tr2[:], in_=tr1[:], axis=mybir.AxisListType.X, op=OP.add)
            nc.scalar.activation(out=lnt[:, 0:1], in_=mv[:, 1:2], func=AF.Ln, bias=eps_t[:])
            if last_v:
                nc.scalar.activation(out=vr2[:], in_=vr2[:], func=AF.Ln, bias=eps_t[:], scale=1.0 / C1)
                nc.scalar.activation(out=tr2[:], in_=tr2[:], func=AF.Ln, bias=eps_t[:], scale=1.0 / C2)
            nc.scalar.activation(out=lnt[:, 1:2], in_=lnt[:, 0:1], func=AF.Exp, scale=-0.5)
            if last_v:
                nc.scalar.activation(out=vr2[:], in_=vr2[:], func=AF.Exp, scale=-0.5)
                nc.scalar.activation(out=tr2[:], in_=tr2[:], func=AF.Exp, scale=-0.5)
            # apply LayerNorm on s
            nc.vector.tensor_scalar(out=snm[:], in0=snm[:], scalar1=mv[:, 0:1],
                                    scalar2=lnt[:, 1:2], op0=OP.subtract, op1=OP.mult)
            nc.vector.tensor_tensor(out=snm[:], in0=snm[:], in1=lw["lngs"][:], op=OP.mult)
            nc.vector.tensor_tensor(out=snm[:], in0=snm[:], in1=lw["lnbs"][:], op=OP.add)
            if last_v:
                nc.vector.tensor_scalar(out=vnm[:], in0=vnm[:], scalar1=vr2[:],
                                        scalar2=None, op0=OP.mult)
                nc.vector.tensor_tensor(
                    out=vnm[:], in0=vnm[:],
                    in1=reap(lw["lngv"][:, 0:1], [(1, C1), (0, 3)]), op=OP.mult)
                nc.vector.tensor_scalar(out=tnm[:], in0=tnm[:], scalar1=tr2[:],
                                        scalar2=None, op0=OP.mult)
                nc.vector.tensor_tensor(
                    out=tnm[:], in0=tnm[:],
                    in1=reap(lw["lngt"][:, 0:1], [(1, C2), (0, 5)]), op=OP.mult)

            # write back feature-major stores
            psn = transpose_to(snm[:], P, P)
            nc.scalar.copy(out=sfm[:, tsl], in_=psn[:, :P])
            if last_v:
                for i in range(3):
                    psn = transpose_to(reap(vnm[:, 0:1, i:i + 1], [(3, C1)]), P, C1)
                    nc.scalar.copy(out=vfm(i)[:, tsl], in_=psn[:C1, :P])
                for m in range(5):
                    psn = transpose_to(reap(tnm[:, 0:1, m:m + 1], [(5, C2)]), P, C2)
                    nc.scalar.copy(out=tfm(m)[:, tsl], in_=psn[:C2, :P])

            if last_v:
                # next-layer node-table projections -> ntrow (node-major, bf16)
                ntrow = poolu.tile([P, GW], BF16, tag="ntrow")

                def proj_to_row(lhsT, rhs, rows, dims, off):
                    ps = psA.tile([P, P], F32, tag="mmA", space="PSUM")
                    nc.tensor.matmul(ps[:rows, :P], lhsT, rhs, start=True, stop=True)
                    sb = poolu.tile([P, P], BF16, tag="projsb")
                    nc.scalar.copy(out=sb[:rows, :P], in_=ps[:rows, :P])
                    psn2 = psA.tile([P, P], BF16, tag="mmA", space="PSUM")
                    nc.tensor.transpose(psn2[:P, :rows], sb[:rows, :P],
                                        ident_t[:rows, :rows])
                    nc.vector.tensor_copy(
                        out=reap(ntrow[:, off:off + 1], dims), in_=psn2[:P, :rows])

                proj_to_row(lw["ws_src2"][:], sfm[:, tsl], C0, [(BLK, 4), (1, 32)], O_S)
                proj_to_row(lw["ws_v2"][:], sfm[:, tsl], C1, [(BLK, 4), (1, 16)], O_SV)
                proj_to_row(lw["ws_t2"][:], sfm[:, tsl], C2, [(BLK, 4), (1, 8)], O_ST)
                for i in range(3):
                    proj_to_row(lw["wv_v2"][:], vfm(i)[:, tsl], C1,
                                [(BLK, 4), (1, 16)], O_V + 16 * i)
                for m in range(5):
                    proj_to_row(lw["wt_t2"][:], tfm(m)[:, tsl], C2,
                                [(BLK, 4), (1, 8)], O_T + 8 * m)
                nc.sync.dma_start(out=own[l][tsl, 0:GW], in_=ntrow[:])
            else:
                # final readout head for this tile
                feat = poolu.tile([P, 4, P], BF16, tag="feat")
                for b in range(4):
                    ps = psA.tile([P, P], F32, tag="mmA", space="PSUM")
                    nc.tensor.matmul(ps[:], wfeat_t[:, b * 128:(b + 1) * 128],
                                     sfm[:, tsl], start=True, stop=True)
                    nc.scalar.activation(out=feat[:, b, :], in_=ps[:],
                                         func=AF.Gelu_apprx_tanh, bias=bfeatp_t[:, b:b + 1])
                pse = psA.tile([1, P], F32, tag="mmA", space="PSUM")
                for b in range(4):
                    nc.tensor.matmul(pse[:], wout1p_t[:, b:b + 1], feat[:, b, :],
                                     start=(b == 0), stop=(b == 3))
                ne = poolu.tile([1, P], F32, tag="ne")
                nc.vector.tensor_copy(out=ne[:], in_=pse[:])
                nc.sync.dma_start(out=t_nodee[tsl], in_=ne[0:1, :])

        for l in range(LYR):
            gsrc = t_ntab0 if l == 0 else ntab[l - 1]
            lw = dict(
                wrad1=loadL(t_wrad1, l, NB, RAD, "wrad1", BF16),
                wrad2=loadL(t_wrad2, l, RAD, RAD, "wrad2", BF16),
                brad1=loadL(t_brad1, l, RAD, 1, "brad1"),
                brad2=loadL(t_brad2, l, RAD, 1, "brad2"),
                wwa=loadL(t_wwa, l, RAD, AW, "wwa", BF16),
                wwb=loadL(t_wwb, l, RAD, BW, "wwb", BF16),
                wo_s=loadL(t_wo_s, l, C0, C0, "wo_s", BF16),
                wo_v=loadL(t_wo_v, l, C1, C1, "wo_v", BF16),
                wo_t=loadL(t_wo_t, l, C2, C2, "wo_t", BF16),
                lngs=loadL(t_lngs, l, P, C0, "lngs", BF16),
                lnbs=loadL(t_lnbs, l, P, C0, "lnbs", BF16),
                lngv=loadL(t_lngv, l, P, C1, "lngv", BF16),
                lngt=loadL(t_lngt, l, P, C2, "lngt", BF16),
            )
            if l < LYR - 1:
                lw["ws_src2"] = loadL(t_ws_src, l + 1, C0, C0, "ws_src2", BF16)
                lw["ws_v2"] = loadL(t_ws_v, l + 1, C0, C1, "ws_v2", BF16)
                lw["ws_t2"] = loadL(t_ws_t, l + 1, C0, C2, "ws_t2", BF16)
                lw["wv_v2"] = loadL(t_wv_v, l + 1, C1, C1, "wv_v2", BF16)
                lw["wt_t2"] = loadL(t_wt_t, l + 1, C2, C2, "wt_t2", BF16)
            for t in range(ntile):
                em_s = poolT.tile([P, cpt * 4, EMW], F32, tag="em_s")
                nc.sync.dma_start(out=em_s[:], in_=t_em[:, t * cpt * 4:(t + 1) * cpt * 4, :])
                idx_s = poolT.tile([P, cpt * 4], mybir.dt.int32, tag="idx_s")
                nc.sync.dma_start(out=idx_s[:], in_=t_idxw[:, t * cpt * 4:(t + 1) * cpt * 4])
                agg = edge_tile(l, t, gsrc, lw, (em_s, idx_s, t * cpt))
                update_tile(l, t, agg, lw)
            if l < LYR - 1:
                nc.gpsimd.collective_compute(
                    "AllGather", OP.bypass,
                    replica_groups=[list(range(cfg.ncore))],
                    ins=[own[l][:]], outs=[ntab[l][:]])

    nc.compile()
    return nc


# ---------------- entry point ----------------

def _ensure_profile_hook():
    try:
        import antenv  # noqa
        import antenv.axon_hooks  # noqa
        return
    except Exception:
        pass
    try:
        import antenv
        from trn_agent_boot.trn_boot import _ntff_profile_via_ctypes
        hook = _ntff_profile_via_ctypes("/opt/axon/libaxon_pjrt.so")
        mod = types.ModuleType("antenv.axon_hooks")
        mod.get_axon_ntff_profile_hook = lambda: hook
        mod.set_axon_ntff_profile_hook = lambda h: None
        sys.modules["antenv.axon_hooks"] = mod
        antenv.axon_hooks = mod
    except Exception:
        pass


_PROGRAM_CACHE = {}


def run_cfg(inp, cfg: Cfg, trace=False):
    in_maps, pid_of = host_preprocess(inp, cfg)
    key = (cfg.ncore, cfg.npc, cfg.cpt, cfg.layers)
    if key not in _PROGRAM_CACHE:
        _PROGRAM_CACHE[key] = build_program(cfg)
    nc = _PROGRAM_CACHE[key]
    if trace:
        _ensure_profile_hook()
    res = run_bass_kernel_spmd(nc, in_maps, list(range(cfg.ncore)), trace=trace)
    node_e_p = np.concatenate(
        [res.results[c]["node_e"] for c in range(cfg.ncore)])
    node_e = node_e_p[pid_of]          # back to original node order
    return node_e, res


def kernel(**inputs):
    cfg = Cfg()
    node_e, _ = run_cfg(inputs, cfg)
    node_e = node_e[:, None] + np.asarray(inputs["b_out1"], np.float32)[None, :]
    batch = np.asarray(inputs["batch"]).astype(np.int64)
    graph = np.zeros((G, 1), np.float32)
    np.add.at(graph, batch, node_e)
    out = graph @ np.asarray(inputs["W_read"], np.float32) + np.asarray(
        inputs["b_read"], np.float32)
    return out.astype(np.float32)
